# revision 8
# baseline (speedup 1.0000x reference)
"""Multi-head causal attention (B=4, T=2048, C=1024, H=16) on 8 TRN2 NeuronCores.

Sharding: data-parallel over batch (4) x tensor-parallel over heads (2 groups
of 8 heads). Core c handles batch c%4, head-group c//4.  Per core:
  - QKV projection in transposed layout (bf16 matmuls, full PE rate).
  - V^T -> V natural layout on the DMA XBAR transpose engine (PE stays free).
  - Causal flash-style attention per head-pair: S^T = K^T.T @ Q^T as a
    row-tiled concurrent pair, E = exp(S^T) on ScalarE, AV and the ones-matmul
    denominator as col-tiled concurrent pairs accumulated over key tiles.
  - Attention is exp-paced, so the remaining QKV m-tiles and the row-parallel
    out-projection are emitted as fine-grained fillers between attention
    k-steps to keep the PE busy during the exp waits.  Units run with big
    query-blocks first so out-projections unblock early.
  - Host adds the two head-group partial [T, C] outputs (out bias is added on
    head-group-0 cores only).
"""

import os
import sys

sys.path.insert(0, "/opt/trn_rl_repo")

from collections import deque

import numpy as np
import ml_dtypes

import concourse.bacc as bacc
import concourse.tile as tile
from concourse import mybir
from concourse.bass_utils import run_bass_kernel_spmd

B, T, C, H, D = 4, 2048, 1024, 16, 64
HPC = 8          # heads per core
PAIRS = HPC // 2
CT = C // 128    # 8 contraction tiles for the projections
MT = 12          # qkv m-tiles per core (4 pairs x {q,k,v})
NQB = T // 512   # 4 query blocks of 512
NKT = T // 128   # 16 key tiles of 128

F32 = mybir.dt.float32
BF16 = mybir.dt.bfloat16

PHASE_NS = 214.0   # one N=512 matmul at 2.4 GHz
PUMP_COEF = 0.45   # filler ns pumped per ns of exp work

LAST_RESULT = None  # stashed BassKernelResults for test harnesses


def build():
    nc = bacc.Bacc("TRN2", target_bir_lowering=False)

    xT = nc.dram_tensor("xT", [C, T], BF16, kind="ExternalInput")
    wtile = nc.dram_tensor("wtile", [128, MT, CT, 128], BF16, kind="ExternalInput")
    bqkv = nc.dram_tensor("bqkv", [128, MT], F32, kind="ExternalInput")
    woT = nc.dram_tensor("woT", [512, C], BF16, kind="ExternalInput")
    bo = nc.dram_tensor("bo", [128, C], F32, kind="ExternalInput")
    maskband = nc.dram_tensor("maskband", [128, NKT, 512], BF16, kind="ExternalInput")
    out = nc.dram_tensor("out", [T, C], BF16, kind="ExternalOutput")

    with tile.TileContext(nc) as tc:
        with tc.tile_pool(name="persist", bufs=1) as pp, \
             tc.tile_pool(name="stream", bufs=2) as sp, \
             tc.tile_pool(name="pss", bufs=2, space="PSUM") as pss, \
             tc.tile_pool(name="psav", bufs=1, space="PSUM") as psav, \
             tc.tile_pool(name="psden", bufs=1, space="PSUM") as psden:

            # -------- weight DMAs for the first m-tiles go out first --------
            wm_tiles = {}

            def fetch_wm(mt):
                if mt in wm_tiles or mt >= MT:
                    return
                wm = sp.tile([128, CT, 128], BF16, tag="wm", bufs=4, name=f"wm{mt}")
                nc.sync.dma_start(out=wm, in_=wtile[:, mt, :, :])
                wm_tiles[mt] = wm

            fetch_wm(0)
            fetch_wm(1)
            fetch_wm(2)
            xt_sb = []
            for ct in range(CT):
                t_ = pp.tile([128, T], BF16, tag="xt", bufs=CT, name=f"xt{ct}")
                # halves alternate between the two hardware DGE queues
                eng = nc.sync if ct % 2 == 0 else nc.scalar
                eng.dma_start(out=t_[:, 0:1024],
                              in_=xT[ct * 128:(ct + 1) * 128, 0:1024])
                xt_sb.append(t_)
            for ct in range(CT):
                eng = nc.sync if ct % 2 == 0 else nc.scalar
                eng.dma_start(out=xt_sb[ct][:, 1024:2048],
                              in_=xT[ct * 128:(ct + 1) * 128, 1024:2048])

            # ---------------- other constants / persistent inputs ----------------
            ones_sb = pp.tile([128, 64], BF16, tag="ones", name="ones")
            nc.vector.memset(ones_sb, 1.0)
            bqkv_sb = pp.tile([128, MT], F32, tag="bqkv", name="bqkv_sb")
            nc.sync.dma_start(out=bqkv_sb, in_=bqkv[:, :])
            bo_sb = pp.tile([128, C], F32, tag="bo", name="bo_sb")
            nc.sync.dma_start(out=bo_sb, in_=bo[:, :])
            mask_sb = pp.tile([128, NKT, 512], BF16, tag="mask", name="mask_sb")
            nc.sync.dma_start(out=mask_sb, in_=maskband[:, :, :])
            wo_sb = []
            for it in range(4):
                w = pp.tile([128, C], BF16, tag="wo", bufs=4, name=f"wo{it}")
                nc.sync.dma_start(out=w, in_=woT[it * 128:(it + 1) * 128, :])
                wo_sb.append(w)

            QT = [pp.tile([128, T], BF16, tag="qt", bufs=4, name=f"qt{p}") for p in range(4)]
            KT = [pp.tile([128, T], BF16, tag="kt", bufs=4, name=f"kt{p}") for p in range(4)]
            # V natural layout: [key-within-tile, ktile, {a dims 64 | b dims 64}]
            V = [pp.tile([128, NKT, 128], BF16, tag="v", bufs=4, name=f"v{p}") for p in range(4)]
            AT = [pp.tile([128, T], BF16, tag="at", bufs=4, name=f"at{p}") for p in range(4)]

            done_mt = set()

            # ---------------- QKV m-tile generator ----------------
            # Matmuls are emitted as (128,64) col-tile pairs: same throughput
            # (the two tiles stream the same rhs concurrently), but the mode
            # matches the attention AV/den matmuls so interleaving them does
            # not force a PE tiling-mode drain.
            def qkv_mtile_gen(mt):
                fetch_wm(mt + 2)   # prefetch two m-tiles ahead
                wm = wm_tiles.pop(mt)
                p, j = divmod(mt, 3)
                if j == 0:
                    dst = QT[p]
                elif j == 1:
                    dst = KT[p]
                else:
                    dst = sp.tile([128, T], BF16, tag="vt", bufs=2, name=f"vt{p}")
                for tch in range(4):
                    ps = pss.tile([128, 512], F32, tag="fill", bufs=2,
                                  name=f"qkv_ps{mt}_{tch}")
                    t0 = tch * 512
                    for ct in range(CT):
                        st, fin = (ct == 0), (ct == CT - 1)
                        xc = xt_sb[ct][:, t0:t0 + 512]
                        nc.tensor.matmul(ps[0:64, :], wm[:, ct, 0:64], xc,
                                         start=st, stop=fin, skip_group_check=True)
                        nc.tensor.matmul(ps[64:128, :], wm[:, ct, 64:128], xc,
                                         start=st, stop=fin, skip_group_check=True)
                        if ct < CT - 1:
                            yield
                    # chunk epilogue rides the last yield so a consumer emitted
                    # right after it never waits on unemitted instructions
                    nc.vector.tensor_scalar_add(
                        dst[:, t0:t0 + 512], ps, bqkv_sb[:, mt:mt + 1])
                    if j == 2:  # V^T -> V natural layout on the DMA XBAR
                        for g in range(tch * 4, tch * 4 + 4):
                            nc.sync.dma_start_transpose(
                                out=V[p][:, g, :],
                                in_=dst[:, g * 128:(g + 1) * 128])
                    yield
                done_mt.add(mt)

            # ---------------- out-projection generator ----------------
            def outproj_gen(tt):
                tq = tt * 128
                o = sp.tile([128, 1024], BF16, tag="o", bufs=3, name=f"o{tt}")
                for oc in range(2):
                    po = pss.tile([128, 512], F32, tag="fill", bufs=2,
                                  name=f"op{tt}_{oc}")
                    for it in range(4):
                        st, fin = (it == 0), (it == 3)
                        wc = wo_sb[it][:, oc * 512:(oc + 1) * 512]
                        nc.tensor.matmul(po[0:64, :], AT[it][:, tq:tq + 64], wc,
                                         start=st, stop=fin, skip_group_check=True)
                        nc.tensor.matmul(po[64:128, :], AT[it][:, tq + 64:tq + 128], wc,
                                         start=st, stop=fin, skip_group_check=True)
                        if it < 3:
                            yield
                    nc.vector.tensor_add(o[:, oc * 512:(oc + 1) * 512], po,
                                         bo_sb[:, oc * 512:(oc + 1) * 512])
                    yield
                nc.sync.dma_start(out=out[tq:tq + 128, :], in_=o)

            # ---------------- filler scheduler ----------------
            filler_q = deque()
            state = {"gen": None, "budget": 0.0}

            def step_filler():
                while True:
                    g = state["gen"]
                    if g is None:
                        if not filler_q:
                            return False
                        state["gen"] = g = filler_q.popleft()
                    try:
                        next(g)
                        return True
                    except StopIteration:
                        state["gen"] = None

            def pump(ns):
                state["budget"] += ns
                while state["budget"] > 0:
                    if not step_filler():
                        state["budget"] = 0.0
                        return
                    state["budget"] -= PHASE_NS

            def need_pair(p):
                mts = {3 * p, 3 * p + 1, 3 * p + 2}
                while not mts <= done_mt:
                    if not step_filler():
                        raise RuntimeError(f"pair {p} never emitted")

            # ---------------- attention unit + out-projection ----------------
            def attn_unit(qb, p, coef=PUMP_COEF):
                nkt = qb * 4 + 4
                q0 = qb * 512
                av = psav.tile([128, 512], F32, tag="av", name=f"av{qb}_{p}")
                den = psden.tile([128, 512], F32, tag="den", name=f"den{qb}_{p}")
                vp = V[p]

                def flush(prev):
                    k, e, off, w = prev
                    st, sp_ = (k == 0), (k == nkt - 1)
                    nc.tensor.matmul(av[0:64, off:off + w],
                                     vp[:, k, 0:64], e[:, 0:w],
                                     start=st, stop=sp_, skip_group_check=True)
                    nc.tensor.matmul(av[64:128, off:off + w],
                                     vp[:, k, 64:128], e[:, 512:512 + w],
                                     start=st, stop=sp_, skip_group_check=True)
                    nc.tensor.matmul(den[0:64, off:off + w], ones_sb, e[:, 0:w],
                                     start=st, stop=sp_, skip_group_check=True)
                    nc.tensor.matmul(den[64:128, off:off + w], ones_sb, e[:, 512:512 + w],
                                     start=st, stop=sp_, skip_group_check=True)

                prev = None
                for k in range(nkt):
                    koff = k - qb * 4
                    # causal: kj-tile k only reaches queries qi >= k*128
                    off = max(koff, 0) * 128
                    w = 512 - off
                    qa = q0 + off
                    ss = pss.tile([128, 1024], F32, tag="ss", bufs=2,
                                  name=f"ss{qb}_{p}_{k}")
                    nc.tensor.matmul(ss[:, 0:w], KT[p][0:64, k * 128:(k + 1) * 128],
                                     QT[p][0:64, qa:qa + w], start=True, stop=True)
                    nc.tensor.matmul(ss[:, 512:512 + w], KT[p][64:128, k * 128:(k + 1) * 128],
                                     QT[p][64:128, qa:qa + w], start=True, stop=True)
                    e = sp.tile([128, 2, 512], BF16, tag="e", bufs=10, name=f"e{qb}_{p}_{k}")
                    nc.scalar.activation(
                        e[:, :, 0:w],
                        ss[:, :].rearrange("a (two n) -> a two n", two=2)[:, :, 0:w],
                        mybir.ActivationFunctionType.Exp)
                    ef = e.rearrange("a two n -> a (two n)")
                    if koff >= 0:
                        # apply the input mask on the leading 128-wide block only:
                        # beyond it every query index exceeds all keys of this tile
                        # (causal tril), so the mask there is all-ones
                        nc.vector.tensor_mul(ef[:, 0:128], ef[:, 0:128],
                                             mask_sb[:, k, off:off + 128])
                        nc.vector.tensor_mul(ef[:, 512:640], ef[:, 512:640],
                                             mask_sb[:, k, off:off + 128])
                    # fill the PE while ScalarE works through exp(k)
                    pump(coef * (2 * w + 352))
                    if prev is not None:
                        flush(prev)
                    prev = (k, ef, off, w)
                flush(prev)

                # copy accumulators out of PSUM first so the banks free early
                avs = sp.tile([128, 512], F32, tag="avs", bufs=4, name=f"avs{qb}_{p}")
                nc.vector.tensor_copy(avs, av)
                rec = sp.tile([128, 512], F32, tag="rec", bufs=6, name=f"rec{qb}_{p}")
                nc.vector.reciprocal_approx_fast(rec, den)
                nc.vector.tensor_mul(AT[p][:, q0:q0 + 512], avs, rec)

            # ------------- emission schedule -------------
            # Prologue: pair 0's projection is emitted token-chunk-wise,
            # interleaved with pair 0's attention units in ascending
            # query-block order — unit (qb,0) only needs tokens 0:(qb+1)*512
            # of Q/K/V, so exp work starts after the first chunks instead of
            # after the whole pair-0 projection.  The remaining m-tiles and
            # unblocked out-projections pump in between attention k-steps.
            qb_done = {qb: 0 for qb in range(4)}

            def op_unblock(qb):
                qb_done[qb] += 1
                if qb_done[qb] == 4:
                    for tt in range(qb * 4, qb * 4 + 4):
                        filler_q.append(outproj_gen(tt))

            for mt in range(3, MT):
                filler_q.append(qkv_mtile_gen(mt))
            gens0 = [qkv_mtile_gen(mt) for mt in (0, 1, 2)]
            for tch in range(4):
                for g in gens0:
                    for _ in range(CT):
                        next(g)
                attn_unit(tch, 0, coef=0.62)
                op_unblock(tch)
            for g in gens0:  # run the generator tails (done_mt bookkeeping)
                for _ in g:
                    pass

            units = [(0, 1), (1, 1), (2, 1), (3, 1),
                     (0, 2), (1, 2), (2, 2), (3, 2),
                     (0, 3), (1, 3), (2, 3), (3, 3)]
            for qb, p in units:
                need_pair(p)
                attn_unit(qb, p)
                op_unblock(qb)
            while step_filler():
                pass

    nc.finalize()
    return nc


_NC = None


def kernel(x, qkv_w, qkv_b, out_w, out_b, attn_mask):
    global _NC, LAST_RESULT
    if _NC is None:
        _NC = build()

    x = np.asarray(x, dtype=np.float32)
    qkv_w = np.asarray(qkv_w, dtype=np.float32)
    qkv_b = np.asarray(qkv_b, dtype=np.float32)
    out_w = np.asarray(out_w, dtype=np.float32)
    out_b = np.asarray(out_b, dtype=np.float32)
    mask = np.asarray(attn_mask).reshape(T, T)

    # mask^T band tiles: band[:, j, :] = mask[qb*512:(qb+1)*512, j*128:(j+1)*128].T
    band = np.empty((128, NKT, 512), dtype=ml_dtypes.bfloat16)
    for j in range(NKT):
        qb = j // 4
        band[:, j, :] = mask[qb * 512:(qb + 1) * 512, j * 128:(j + 1) * 128].astype(
            ml_dtypes.bfloat16).T

    in_maps = []
    for c in range(8):
        b, hg = c % 4, c // 4
        h0 = hg * HPC
        # per-pair [q;k;v] row blocks of qkv_w, transposed; q pre-scaled by 1/8
        blocks = []
        bias_cols = np.empty((128, MT), dtype=np.float32)
        for p in range(PAIRS):
            r0 = (h0 + 2 * p) * D
            qrows = qkv_w[r0:r0 + 128] * 0.125
            krows = qkv_w[C + r0:C + r0 + 128]
            vrows = qkv_w[2 * C + r0:2 * C + r0 + 128]
            blocks += [qrows, krows, vrows]
            bias_cols[:, 3 * p + 0] = qkv_b[r0:r0 + 128] * 0.125
            bias_cols[:, 3 * p + 1] = qkv_b[C + r0:C + r0 + 128]
            bias_cols[:, 3 * p + 2] = qkv_b[2 * C + r0:2 * C + r0 + 128]
        wqkvT = np.concatenate(blocks, axis=0).T  # [C, MT*128]
        wtile = np.ascontiguousarray(
            wqkvT.reshape(CT, 128, MT, 128).transpose(1, 2, 0, 3)).astype(ml_dtypes.bfloat16)
        woT = np.ascontiguousarray(
            out_w[:, h0 * D:(h0 + HPC) * D].T).astype(ml_dtypes.bfloat16)
        bo = (np.tile(out_b, (128, 1)) if hg == 0
              else np.zeros((128, C), np.float32)).astype(np.float32)
        in_maps.append({
            "xT": np.ascontiguousarray(x[b].T).astype(ml_dtypes.bfloat16),
            "wtile": wtile,
            "bqkv": bias_cols,
            "woT": woT,
            "bo": bo,
            "maskband": band,
        })

    LAST_RESULT = run_bass_kernel_spmd(_NC, in_maps, core_ids=list(range(8)))
    res = LAST_RESULT.results
    out = np.empty((B, T, C), dtype=np.float32)
    for b in range(B):
        out[b] = (res[b]["out"].astype(np.float32)
                  + res[b + 4]["out"].astype(np.float32))
    return out


# revision 9
# speedup vs baseline: 1.0067x; 1.0067x over previous
"""Multi-head causal attention (B=4, T=2048, C=1024, H=16) on 8 TRN2 NeuronCores.

Sharding: data-parallel over batch (4) x tensor-parallel over heads (2 groups
of 8 heads). Core c handles batch c%4, head-group c//4.  Per core:
  - QKV projection in transposed layout (bf16 matmuls, full PE rate).
  - V^T -> V natural layout on the DMA XBAR transpose engine (PE stays free).
  - Causal flash-style attention per head-pair: S^T = K^T.T @ Q^T as a
    row-tiled concurrent pair, E = exp(S^T) on ScalarE, AV and the ones-matmul
    denominator as col-tiled concurrent pairs accumulated over key tiles.
  - Attention is exp-paced, so the remaining QKV m-tiles and the row-parallel
    out-projection are emitted as fine-grained fillers between attention
    k-steps to keep the PE busy during the exp waits.  Units run with big
    query-blocks first so out-projections unblock early.
  - Host adds the two head-group partial [T, C] outputs (out bias is added on
    head-group-0 cores only).
"""

import os
import sys

sys.path.insert(0, "/opt/trn_rl_repo")

from collections import deque

import numpy as np
import ml_dtypes

import concourse.bacc as bacc
import concourse.tile as tile
from concourse import mybir
from concourse.bass_utils import run_bass_kernel_spmd

B, T, C, H, D = 4, 2048, 1024, 16, 64
HPC = 8          # heads per core
PAIRS = HPC // 2
CT = C // 128    # 8 contraction tiles for the projections
MT = 12          # qkv m-tiles per core (4 pairs x {q,k,v})
NQB = T // 512   # 4 query blocks of 512
NKT = T // 128   # 16 key tiles of 128

F32 = mybir.dt.float32
BF16 = mybir.dt.bfloat16

PHASE_NS = 214.0   # one N=512 matmul at 2.4 GHz
PUMP_COEF = 0.45   # filler ns pumped per ns of exp work

LAST_RESULT = None  # stashed BassKernelResults for test harnesses


def build():
    nc = bacc.Bacc("TRN2", target_bir_lowering=False)

    xT = nc.dram_tensor("xT", [C, T], BF16, kind="ExternalInput")
    wtile = nc.dram_tensor("wtile", [128, MT, CT, 128], BF16, kind="ExternalInput")
    bqkv = nc.dram_tensor("bqkv", [128, MT], F32, kind="ExternalInput")
    woT = nc.dram_tensor("woT", [512, C], BF16, kind="ExternalInput")
    bo = nc.dram_tensor("bo", [128, C], F32, kind="ExternalInput")
    maskband = nc.dram_tensor("maskband", [128, NKT, 128], BF16, kind="ExternalInput")
    out = nc.dram_tensor("out", [T, C], BF16, kind="ExternalOutput")

    with tile.TileContext(nc) as tc:
        with tc.tile_pool(name="persist", bufs=1) as pp, \
             tc.tile_pool(name="stream", bufs=2) as sp, \
             tc.tile_pool(name="pss", bufs=2, space="PSUM") as pss, \
             tc.tile_pool(name="psav", bufs=1, space="PSUM") as psav, \
             tc.tile_pool(name="psden", bufs=1, space="PSUM") as psden:

            # -------- weight DMAs for the first m-tiles go out first --------
            wm_tiles = {}

            def fetch_wm(mt):
                if mt in wm_tiles or mt >= MT:
                    return
                wm = sp.tile([128, CT, 128], BF16, tag="wm", bufs=4, name=f"wm{mt}")
                nc.sync.dma_start(out=wm, in_=wtile[:, mt, :, :])
                wm_tiles[mt] = wm

            fetch_wm(0)
            bqkv_sb = pp.tile([128, MT], F32, tag="bqkv", name="bqkv_sb")
            nc.sync.dma_start(out=bqkv_sb, in_=bqkv[:, :])
            fetch_wm(1)
            fetch_wm(2)
            # x arrives in four 512-col waves, tiles alternating across the
            # two hardware DGE queues, so the first m-tile chunk computes
            # within ~1us of kernel start and the PE never goes HAM-cold.
            xt_sb = [pp.tile([128, T], BF16, tag="xt", bufs=CT, name=f"xt{ct}")
                     for ct in range(CT)]
            for wv in range(4):
                for ct in range(CT):
                    eng = nc.sync if ct % 2 == 0 else nc.scalar
                    eng.dma_start(out=xt_sb[ct][:, wv * 512:(wv + 1) * 512],
                                  in_=xT[ct * 128:(ct + 1) * 128, wv * 512:(wv + 1) * 512])
                if wv == 1:
                    mask_sb = pp.tile([128, NKT, 128], BF16, tag="mask", name="mask_sb")
                    nc.scalar.dma_start(out=mask_sb, in_=maskband[:, :, :])

            # ---------------- other constants / persistent inputs ----------------
            ones_sb = pp.tile([128, 64], BF16, tag="ones", name="ones")
            nc.vector.memset(ones_sb, 1.0)
            bo_sb = pp.tile([128, C], F32, tag="bo", name="bo_sb")
            nc.scalar.dma_start(out=bo_sb, in_=bo[:, :])
            wo_sb = []
            for it in range(4):
                w = pp.tile([128, C], BF16, tag="wo", bufs=4, name=f"wo{it}")
                eng = nc.sync if it % 2 == 0 else nc.scalar
                eng.dma_start(out=w, in_=woT[it * 128:(it + 1) * 128, :])
                wo_sb.append(w)

            QT = [pp.tile([128, T], BF16, tag="qt", bufs=4, name=f"qt{p}") for p in range(4)]
            KT = [pp.tile([128, T], BF16, tag="kt", bufs=4, name=f"kt{p}") for p in range(4)]
            # V natural layout: [key-within-tile, ktile, {a dims 64 | b dims 64}]
            V = [pp.tile([128, NKT, 128], BF16, tag="v", bufs=4, name=f"v{p}") for p in range(4)]
            AT = [pp.tile([128, T], BF16, tag="at", bufs=4, name=f"at{p}") for p in range(4)]

            done_mt = set()

            # ---------------- QKV m-tile generator ----------------
            # Matmuls are emitted as (128,64) col-tile pairs: same throughput
            # (the two tiles stream the same rhs concurrently), but the mode
            # matches the attention AV/den matmuls so interleaving them does
            # not force a PE tiling-mode drain.
            def qkv_mtile_gen(mt):
                fetch_wm(mt + 2)   # prefetch two m-tiles ahead
                wm = wm_tiles.pop(mt)
                p, j = divmod(mt, 3)
                if j == 0:
                    dst = QT[p]
                elif j == 1:
                    dst = KT[p]
                else:
                    dst = sp.tile([128, T], BF16, tag="vt", bufs=2, name=f"vt{p}")
                for tch in range(4):
                    ps = pss.tile([128, 512], F32, tag="fill", bufs=2,
                                  name=f"qkv_ps{mt}_{tch}")
                    t0 = tch * 512
                    for ct in range(CT):
                        st, fin = (ct == 0), (ct == CT - 1)
                        xc = xt_sb[ct][:, t0:t0 + 512]
                        nc.tensor.matmul(ps[0:64, :], wm[:, ct, 0:64], xc,
                                         start=st, stop=fin, skip_group_check=True)
                        nc.tensor.matmul(ps[64:128, :], wm[:, ct, 64:128], xc,
                                         start=st, stop=fin, skip_group_check=True)
                        if ct < CT - 1:
                            yield
                    # chunk epilogue rides the last yield so a consumer emitted
                    # right after it never waits on unemitted instructions
                    nc.vector.tensor_scalar_add(
                        dst[:, t0:t0 + 512], ps, bqkv_sb[:, mt:mt + 1])
                    if j == 2:  # V^T -> V natural layout on the DMA XBAR
                        for g in range(tch * 4, tch * 4 + 4):
                            nc.sync.dma_start_transpose(
                                out=V[p][:, g, :],
                                in_=dst[:, g * 128:(g + 1) * 128])
                    yield
                done_mt.add(mt)

            # ---------------- out-projection generator ----------------
            def outproj_gen(tt, tail=False):
                tq = tt * 128
                o = sp.tile([128, 1024], BF16, tag="o", bufs=3, name=f"o{tt}")
                for oc in range(2):
                    if tail:
                        pool, tag = ((psav, "av"), (psden, "den"))[(2 * tt + oc) % 2]
                        po = pool.tile([128, 512], F32, tag=tag, name=f"op{tt}_{oc}")
                    else:
                        po = pss.tile([128, 512], F32, tag="fill", bufs=2,
                                      name=f"op{tt}_{oc}")
                    for it in range(4):
                        st, fin = (it == 0), (it == 3)
                        wc = wo_sb[it][:, oc * 512:(oc + 1) * 512]
                        nc.tensor.matmul(po[0:64, :], AT[it][:, tq:tq + 64], wc,
                                         start=st, stop=fin, skip_group_check=True)
                        nc.tensor.matmul(po[64:128, :], AT[it][:, tq + 64:tq + 128], wc,
                                         start=st, stop=fin, skip_group_check=True)
                        if it < 3:
                            yield
                    nc.vector.tensor_add(o[:, oc * 512:(oc + 1) * 512], po,
                                         bo_sb[:, oc * 512:(oc + 1) * 512])
                    yield
                (nc.scalar if tail else nc.sync).dma_start(
                    out=out[tq:tq + 128, :], in_=o)

            # ---------------- filler scheduler ----------------
            filler_q = deque()
            state = {"gen": None, "budget": 0.0}

            def step_filler():
                while True:
                    g = state["gen"]
                    if g is None:
                        if not filler_q:
                            return False
                        state["gen"] = g = filler_q.popleft()
                    try:
                        next(g)
                        return True
                    except StopIteration:
                        state["gen"] = None

            def pump(ns):
                state["budget"] += ns
                while state["budget"] > 0:
                    if not step_filler():
                        state["budget"] = 0.0
                        return
                    state["budget"] -= PHASE_NS

            def need_pair(p):
                mts = {3 * p, 3 * p + 1, 3 * p + 2}
                while not mts <= done_mt:
                    if not step_filler():
                        raise RuntimeError(f"pair {p} never emitted")

            # ---------------- attention unit + out-projection ----------------
            def attn_unit(qb, p, coef=PUMP_COEF):
                nkt = qb * 4 + 4
                q0 = qb * 512
                av = psav.tile([128, 512], F32, tag="av", name=f"av{qb}_{p}")
                den = psden.tile([128, 512], F32, tag="den", name=f"den{qb}_{p}")
                vp = V[p]

                def flush(prev):
                    k, e, off, w = prev
                    st, sp_ = (k == 0), (k == nkt - 1)
                    nc.tensor.matmul(av[0:64, off:off + w],
                                     vp[:, k, 0:64], e[:, 0:w],
                                     start=st, stop=sp_, skip_group_check=True)
                    nc.tensor.matmul(av[64:128, off:off + w],
                                     vp[:, k, 64:128], e[:, 512:512 + w],
                                     start=st, stop=sp_, skip_group_check=True)
                    nc.tensor.matmul(den[0:64, off:off + w], ones_sb, e[:, 0:w],
                                     start=st, stop=sp_, skip_group_check=True)
                    nc.tensor.matmul(den[64:128, off:off + w], ones_sb, e[:, 512:512 + w],
                                     start=st, stop=sp_, skip_group_check=True)

                prev = None
                for k in range(nkt):
                    koff = k - qb * 4
                    # causal: kj-tile k only reaches queries qi >= k*128
                    off = max(koff, 0) * 128
                    w = 512 - off
                    qa = q0 + off
                    ss = pss.tile([128, 1024], F32, tag="ss", bufs=2,
                                  name=f"ss{qb}_{p}_{k}")
                    nc.tensor.matmul(ss[:, 0:w], KT[p][0:64, k * 128:(k + 1) * 128],
                                     QT[p][0:64, qa:qa + w], start=True, stop=True)
                    nc.tensor.matmul(ss[:, 512:512 + w], KT[p][64:128, k * 128:(k + 1) * 128],
                                     QT[p][64:128, qa:qa + w], start=True, stop=True)
                    e = sp.tile([128, 2, 512], BF16, tag="e", bufs=10, name=f"e{qb}_{p}_{k}")
                    nc.scalar.activation(
                        e[:, :, 0:w],
                        ss[:, :].rearrange("a (two n) -> a two n", two=2)[:, :, 0:w],
                        mybir.ActivationFunctionType.Exp)
                    ef = e.rearrange("a two n -> a (two n)")
                    if koff >= 0:
                        # apply the input mask on the leading 128-wide block only:
                        # beyond it every query index exceeds all keys of this tile
                        # (causal tril), so the mask there is all-ones
                        nc.vector.tensor_mul(ef[:, 0:128], ef[:, 0:128],
                                             mask_sb[:, k, :])
                        nc.vector.tensor_mul(ef[:, 512:640], ef[:, 512:640],
                                             mask_sb[:, k, :])
                    # fill the PE while ScalarE works through exp(k)
                    pump(coef * (2 * w + 352))
                    if prev is not None:
                        flush(prev)
                    prev = (k, ef, off, w)
                flush(prev)

                # copy accumulators out of PSUM first so the banks free early
                avs = sp.tile([128, 512], F32, tag="avs", bufs=4, name=f"avs{qb}_{p}")
                nc.vector.tensor_copy(avs, av)
                rec = sp.tile([128, 512], F32, tag="rec", bufs=6, name=f"rec{qb}_{p}")
                nc.vector.reciprocal_approx_fast(rec, den)
                nc.vector.tensor_mul(AT[p][:, q0:q0 + 512], avs, rec)

            # ------------- emission schedule -------------
            # Prologue: pair 0's projection is emitted token-chunk-wise,
            # interleaved with pair 0's attention units in ascending
            # query-block order — unit (qb,0) only needs tokens 0:(qb+1)*512
            # of Q/K/V, so exp work starts after the first chunks instead of
            # after the whole pair-0 projection.  The remaining m-tiles and
            # unblocked out-projections pump in between attention k-steps.
            qb_done = {qb: 0 for qb in range(4)}

            def op_unblock(qb, tail=False):
                qb_done[qb] += 1
                if qb_done[qb] == 4:
                    for tt in range(qb * 4, qb * 4 + 4):
                        filler_q.append(outproj_gen(tt, tail=tail))

            for mt in range(3, MT):
                filler_q.append(qkv_mtile_gen(mt))
            gens0 = [qkv_mtile_gen(mt) for mt in (0, 1, 2)]
            for tch in range(4):
                for g in gens0:
                    for _ in range(CT):
                        next(g)
                attn_unit(tch, 0, coef=0.62)
                op_unblock(tch)
            for g in gens0:  # run the generator tails (done_mt bookkeeping)
                for _ in g:
                    pass

            units = [(0, 1), (1, 1), (2, 1), (3, 1),
                     (0, 2), (1, 2), (2, 2), (3, 2),
                     (0, 3), (1, 3), (2, 3), (3, 3)]
            for i, (qb, p) in enumerate(units):
                need_pair(p)
                attn_unit(qb, p)
                op_unblock(qb, tail=(i == len(units) - 1))
            while step_filler():
                pass

    nc.finalize()
    return nc


_NC = None


def kernel(x, qkv_w, qkv_b, out_w, out_b, attn_mask):
    global _NC, LAST_RESULT
    if _NC is None:
        _NC = build()

    x = np.asarray(x, dtype=np.float32)
    qkv_w = np.asarray(qkv_w, dtype=np.float32)
    qkv_b = np.asarray(qkv_b, dtype=np.float32)
    out_w = np.asarray(out_w, dtype=np.float32)
    out_b = np.asarray(out_b, dtype=np.float32)
    mask = np.asarray(attn_mask).reshape(T, T)

    # only the diagonal 128-block of each key tile is ever masked
    band = np.empty((128, NKT, 128), dtype=ml_dtypes.bfloat16)
    for j in range(NKT):
        band[:, j, :] = mask[j * 128:(j + 1) * 128, j * 128:(j + 1) * 128].astype(
            ml_dtypes.bfloat16).T

    in_maps = []
    for c in range(8):
        b, hg = c % 4, c // 4
        h0 = hg * HPC
        # per-pair [q;k;v] row blocks of qkv_w, transposed; q pre-scaled by 1/8
        blocks = []
        bias_cols = np.empty((128, MT), dtype=np.float32)
        for p in range(PAIRS):
            r0 = (h0 + 2 * p) * D
            qrows = qkv_w[r0:r0 + 128] * 0.125
            krows = qkv_w[C + r0:C + r0 + 128]
            vrows = qkv_w[2 * C + r0:2 * C + r0 + 128]
            blocks += [qrows, krows, vrows]
            bias_cols[:, 3 * p + 0] = qkv_b[r0:r0 + 128] * 0.125
            bias_cols[:, 3 * p + 1] = qkv_b[C + r0:C + r0 + 128]
            bias_cols[:, 3 * p + 2] = qkv_b[2 * C + r0:2 * C + r0 + 128]
        wqkvT = np.concatenate(blocks, axis=0).T  # [C, MT*128]
        wtile = np.ascontiguousarray(
            wqkvT.reshape(CT, 128, MT, 128).transpose(1, 2, 0, 3)).astype(ml_dtypes.bfloat16)
        woT = np.ascontiguousarray(
            out_w[:, h0 * D:(h0 + HPC) * D].T).astype(ml_dtypes.bfloat16)
        bo = (np.tile(out_b, (128, 1)) if hg == 0
              else np.zeros((128, C), np.float32)).astype(np.float32)
        in_maps.append({
            "xT": np.ascontiguousarray(x[b].T).astype(ml_dtypes.bfloat16),
            "wtile": wtile,
            "bqkv": bias_cols,
            "woT": woT,
            "bo": bo,
            "maskband": band,
        })

    LAST_RESULT = run_bass_kernel_spmd(_NC, in_maps, core_ids=list(range(8)))
    res = LAST_RESULT.results
    out = np.empty((B, T, C), dtype=np.float32)
    for b in range(B):
        out[b] = (res[b]["out"].astype(np.float32)
                  + res[b + 4]["out"].astype(np.float32))
    return out


# revision 10
# speedup vs baseline: 1.0829x; 1.0757x over previous
"""Multi-head causal attention (B=4, T=2048, C=1024, H=16) on 8 TRN2 NeuronCores.

Sharding: data-parallel over batch (4) x tensor-parallel over heads (2 groups
of 8 heads). Core c handles batch c%4, head-group c//4.  Per core:
  - QKV projection in transposed layout (bf16 matmuls, full PE rate).
  - V^T -> V natural layout on the DMA XBAR transpose engine (PE stays free).
  - Causal flash-style attention per head-pair: S^T = K^T.T @ Q^T as a
    row-tiled concurrent pair, E = exp(S^T) on ScalarE, AV and the ones-matmul
    denominator as col-tiled concurrent pairs accumulated over key tiles.
  - Attention is exp-paced, so the remaining QKV m-tiles and the row-parallel
    out-projection are emitted as fine-grained fillers between attention
    k-steps to keep the PE busy during the exp waits.  Units run with big
    query-blocks first so out-projections unblock early.
  - Host adds the two head-group partial [T, C] outputs (out bias is added on
    head-group-0 cores only).
"""

import os
import sys

sys.path.insert(0, "/opt/trn_rl_repo")

from collections import deque

import numpy as np
import ml_dtypes

import concourse.bacc as bacc
import concourse.tile as tile
from concourse import mybir
from concourse.bass_utils import run_bass_kernel_spmd

B, T, C, H, D = 4, 2048, 1024, 16, 64
HPC = 8          # heads per core
PAIRS = HPC // 2
CT = C // 128    # 8 contraction tiles for the projections
MT = 12          # qkv m-tiles per core (4 pairs x {q,k,v})
NQB = T // 512   # 4 query blocks of 512
NKT = T // 128   # 16 key tiles of 128

F32 = mybir.dt.float32
BF16 = mybir.dt.bfloat16

PHASE_NS = 214.0   # one N=512 matmul at 2.4 GHz
PUMP_COEF = 0.45   # filler ns pumped per ns of exp work

LAST_RESULT = None  # stashed BassKernelResults for test harnesses


def build():
    nc = bacc.Bacc("TRN2", target_bir_lowering=False)

    xT = nc.dram_tensor("xT", [C, T], BF16, kind="ExternalInput")
    wtile = nc.dram_tensor("wtile", [128, MT, CT, 128], BF16, kind="ExternalInput")
    bqkv = nc.dram_tensor("bqkv", [128, MT], F32, kind="ExternalInput")
    woT = nc.dram_tensor("woT", [512, C], BF16, kind="ExternalInput")
    bo = nc.dram_tensor("bo", [128, C], F32, kind="ExternalInput")
    maskband = nc.dram_tensor("maskband", [128, NKT, 128], BF16, kind="ExternalInput")
    out = nc.dram_tensor("out", [T, C], BF16, kind="ExternalOutput")

    with tile.TileContext(nc) as tc:
        with tc.tile_pool(name="persist", bufs=1) as pp, \
             tc.tile_pool(name="stream", bufs=2) as sp, \
             tc.tile_pool(name="pss", bufs=2, space="PSUM") as pss, \
             tc.tile_pool(name="psav", bufs=1, space="PSUM") as psav, \
             tc.tile_pool(name="psden", bufs=1, space="PSUM") as psden:

            # -------- weight DMAs for the first m-tiles go out first --------
            wm_tiles = {}

            def fetch_wm(mt):
                if mt in wm_tiles or mt >= MT:
                    return
                wm = sp.tile([128, CT, 128], BF16, tag="wm", bufs=4, name=f"wm{mt}")
                nc.sync.dma_start(out=wm, in_=wtile[:, mt, :, :])
                wm_tiles[mt] = wm

            fetch_wm(0)
            bqkv_sb = pp.tile([128, MT], F32, tag="bqkv", name="bqkv_sb")
            nc.sync.dma_start(out=bqkv_sb, in_=bqkv[:, :])
            fetch_wm(1)
            fetch_wm(2)
            # x arrives in four 512-col waves, tiles alternating across the
            # two hardware DGE queues, so the first m-tile chunk computes
            # within ~1us of kernel start and the PE never goes HAM-cold.
            xt_sb = [pp.tile([128, T], BF16, tag="xt", bufs=CT, name=f"xt{ct}")
                     for ct in range(CT)]
            for wv in range(4):
                for ct in range(CT):
                    eng = nc.sync if ct % 2 == 0 else nc.scalar
                    eng.dma_start(out=xt_sb[ct][:, wv * 512:(wv + 1) * 512],
                                  in_=xT[ct * 128:(ct + 1) * 128, wv * 512:(wv + 1) * 512])
                if wv == 1:
                    mask_sb = pp.tile([128, NKT, 128], BF16, tag="mask", name="mask_sb")
                    nc.scalar.dma_start(out=mask_sb, in_=maskband[:, :, :])

            # ---------------- other constants / persistent inputs ----------------
            ones_sb = pp.tile([128, 64], BF16, tag="ones", name="ones")
            nc.vector.memset(ones_sb, 1.0)
            bo_sb = pp.tile([128, C], F32, tag="bo", name="bo_sb")
            nc.scalar.dma_start(out=bo_sb, in_=bo[:, :])
            wo_sb = []
            for it in range(4):
                w = pp.tile([128, C], BF16, tag="wo", bufs=4, name=f"wo{it}")
                eng = nc.sync if it % 2 == 0 else nc.scalar
                eng.dma_start(out=w, in_=woT[it * 128:(it + 1) * 128, :])
                wo_sb.append(w)

            QT = [pp.tile([128, T], BF16, tag="qt", bufs=4, name=f"qt{p}") for p in range(4)]
            KT = [pp.tile([128, T], BF16, tag="kt", bufs=4, name=f"kt{p}") for p in range(4)]
            # V natural layout: [key-within-tile, ktile, {a dims 64 | b dims 64}]
            V = [pp.tile([128, NKT, 128], BF16, tag="v", bufs=4, name=f"v{p}") for p in range(4)]
            AT = [pp.tile([128, T], BF16, tag="at", bufs=4, name=f"at{p}") for p in range(4)]

            done_mt = set()

            # ---------------- QKV m-tile generator ----------------
            # Matmuls are emitted as (128,64) col-tile pairs: same throughput
            # (the two tiles stream the same rhs concurrently), but the mode
            # matches the attention AV/den matmuls so interleaving them does
            # not force a PE tiling-mode drain.
            def qkv_mtile_gen(mt):
                fetch_wm(mt + 2)   # prefetch two m-tiles ahead
                wm = wm_tiles.pop(mt)
                p, j = divmod(mt, 3)
                if j == 0:
                    dst = QT[p]
                elif j == 1:
                    dst = KT[p]
                else:
                    dst = sp.tile([128, T], BF16, tag="vt", bufs=2, name=f"vt{p}")
                for tch in range(4):
                    ps = pss.tile([128, 512], F32, tag="fill", bufs=2,
                                  name=f"qkv_ps{mt}_{tch}")
                    t0 = tch * 512
                    for ct in range(CT):
                        st, fin = (ct == 0), (ct == CT - 1)
                        xc = xt_sb[ct][:, t0:t0 + 512]
                        nc.tensor.matmul(ps[0:64, :], wm[:, ct, 0:64], xc,
                                         start=st, stop=fin, skip_group_check=True)
                        nc.tensor.matmul(ps[64:128, :], wm[:, ct, 64:128], xc,
                                         start=st, stop=fin, skip_group_check=True)
                        if ct < CT - 1:
                            yield
                    # chunk epilogue rides the last yield so a consumer emitted
                    # right after it never waits on unemitted instructions
                    nc.vector.tensor_scalar_add(
                        dst[:, t0:t0 + 512], ps, bqkv_sb[:, mt:mt + 1])
                    if j == 2:  # V^T -> V natural layout on the DMA XBAR
                        for g in range(tch * 4, tch * 4 + 4):
                            nc.sync.dma_start_transpose(
                                out=V[p][:, g, :],
                                in_=dst[:, g * 128:(g + 1) * 128])
                    yield
                done_mt.add(mt)

            # ---------------- out-projection generator ----------------
            def outproj_gen(tt, tail=False):
                tq = tt * 128
                o = sp.tile([128, 1024], BF16, tag="o", bufs=3, name=f"o{tt}")
                for oc in range(2):
                    if tail:
                        pool, tag = ((psav, "av"), (psden, "den"))[(2 * tt + oc) % 2]
                        po = pool.tile([128, 512], F32, tag=tag, name=f"op{tt}_{oc}")
                    else:
                        po = pss.tile([128, 512], F32, tag="fill", bufs=2,
                                      name=f"op{tt}_{oc}")
                    for it in range(4):
                        st, fin = (it == 0), (it == 3)
                        wc = wo_sb[it][:, oc * 512:(oc + 1) * 512]
                        nc.tensor.matmul(po[0:64, :], AT[it][:, tq:tq + 64], wc,
                                         start=st, stop=fin, skip_group_check=True)
                        nc.tensor.matmul(po[64:128, :], AT[it][:, tq + 64:tq + 128], wc,
                                         start=st, stop=fin, skip_group_check=True)
                        if it < 3:
                            yield
                    nc.vector.tensor_add(o[:, oc * 512:(oc + 1) * 512], po,
                                         bo_sb[:, oc * 512:(oc + 1) * 512])
                    yield
                (nc.scalar if tail else nc.sync).dma_start(
                    out=out[tq:tq + 128, :], in_=o)

            # ---------------- filler scheduler ----------------
            filler_q = deque()
            state = {"gen": None, "budget": 0.0}

            def step_filler():
                while True:
                    g = state["gen"]
                    if g is None:
                        if not filler_q:
                            return False
                        state["gen"] = g = filler_q.popleft()
                    try:
                        next(g)
                        return True
                    except StopIteration:
                        state["gen"] = None

            def pump(ns):
                state["budget"] += ns
                while state["budget"] > 0:
                    if not step_filler():
                        state["budget"] = 0.0
                        return
                    state["budget"] -= PHASE_NS

            def need_pair(p):
                mts = {3 * p, 3 * p + 1, 3 * p + 2}
                while not mts <= done_mt:
                    if not step_filler():
                        raise RuntimeError(f"pair {p} never emitted")

            # ---------------- attention unit + out-projection ----------------
            def attn_unit(qb, p, coef=PUMP_COEF):
                nkt = qb * 4 + 4
                q0 = qb * 512
                av = psav.tile([128, 512], F32, tag="av", name=f"av{qb}_{p}")
                den = psden.tile([128, 512], F32, tag="den", name=f"den{qb}_{p}")
                vp = V[p]

                def flush(prev):
                    k, e, off, w = prev
                    st, sp_ = (k == 0), (k == nkt - 1)
                    nc.tensor.matmul(av[0:64, off:off + w],
                                     vp[:, k, 0:64], e[:, 0:w],
                                     start=st, stop=sp_, skip_group_check=True)
                    nc.tensor.matmul(av[64:128, off:off + w],
                                     vp[:, k, 64:128], e[:, 512:512 + w],
                                     start=st, stop=sp_, skip_group_check=True)
                    nc.tensor.matmul(den[0:64, off:off + w], ones_sb, e[:, 0:w],
                                     start=st, stop=sp_, skip_group_check=True)
                    nc.tensor.matmul(den[64:128, off:off + w], ones_sb, e[:, 512:512 + w],
                                     start=st, stop=sp_, skip_group_check=True)

                # k-tiles are processed in pairs: the two S row-pairs are
                # emitted back-to-back (one tiling-mode round trip per pair
                # instead of two), and AV/den flushes run one k-pair behind so
                # the exp->flush handoff semaphore has long since fired.
                pending = deque()
                for kk in range(0, nkt, 2):
                    items = []
                    for k in (kk, kk + 1):
                        koff = k - qb * 4
                        # causal: kj-tile k only reaches queries qi >= k*128
                        off = max(koff, 0) * 128
                        w = 512 - off
                        qa = q0 + off
                        ss = pss.tile([128, 1024], F32, tag="ss", bufs=2,
                                      name=f"ss{qb}_{p}_{k}")
                        nc.tensor.matmul(ss[:, 0:w], KT[p][0:64, k * 128:(k + 1) * 128],
                                         QT[p][0:64, qa:qa + w], start=True, stop=True)
                        nc.tensor.matmul(ss[:, 512:512 + w], KT[p][64:128, k * 128:(k + 1) * 128],
                                         QT[p][64:128, qa:qa + w], start=True, stop=True)
                        items.append([k, ss, None, off, w])
                    for it_ in items:
                        k, ss, _, off, w = it_
                        e = sp.tile([128, 2, 512], BF16, tag="e", bufs=10,
                                    name=f"e{qb}_{p}_{k}")
                        nc.scalar.activation(
                            e[:, :, 0:w],
                            ss[:, :].rearrange("a (two n) -> a two n", two=2)[:, :, 0:w],
                            mybir.ActivationFunctionType.Exp)
                        ef = e.rearrange("a two n -> a (two n)")
                        if k - qb * 4 >= 0:
                            # mask only the leading 128-wide diagonal block:
                            # beyond it every query index exceeds all keys of
                            # this tile (causal tril), so the mask is all-ones
                            nc.vector.tensor_mul(ef[:, 0:128], ef[:, 0:128],
                                                 mask_sb[:, k, :])
                            nc.vector.tensor_mul(ef[:, 512:640], ef[:, 512:640],
                                                 mask_sb[:, k, :])
                        it_[2] = ef
                    # fill the PE while ScalarE works through the two exps
                    pump(coef * sum(2 * it_[4] + 352 for it_ in items))
                    pending.append(items)
                    if len(pending) > 1:
                        for k, _, ef, off, w in pending.popleft():
                            flush((k, ef, off, w))
                while pending:
                    for k, _, ef, off, w in pending.popleft():
                        flush((k, ef, off, w))

                # copy accumulators out of PSUM first so the banks free early
                avs = sp.tile([128, 512], F32, tag="avs", bufs=4, name=f"avs{qb}_{p}")
                nc.vector.tensor_copy(avs, av)
                rec = sp.tile([128, 512], F32, tag="rec", bufs=6, name=f"rec{qb}_{p}")
                nc.vector.reciprocal_approx_fast(rec, den)
                nc.vector.tensor_mul(AT[p][:, q0:q0 + 512], avs, rec)

            # ------------- emission schedule -------------
            # Prologue: pair 0's projection is emitted token-chunk-wise,
            # interleaved with pair 0's attention units in ascending
            # query-block order — unit (qb,0) only needs tokens 0:(qb+1)*512
            # of Q/K/V, so exp work starts after the first chunks instead of
            # after the whole pair-0 projection.  The remaining m-tiles and
            # unblocked out-projections pump in between attention k-steps.
            qb_done = {qb: 0 for qb in range(4)}

            def op_unblock(qb, tail=False):
                qb_done[qb] += 1
                if qb_done[qb] == 4:
                    for tt in range(qb * 4, qb * 4 + 4):
                        filler_q.append(outproj_gen(tt, tail=tail))

            for mt in range(3, MT):
                filler_q.append(qkv_mtile_gen(mt))
            gens0 = [qkv_mtile_gen(mt) for mt in (0, 1, 2)]
            for tch in range(4):
                for g in gens0:
                    for _ in range(CT):
                        next(g)
                attn_unit(tch, 0, coef=0.62)
                op_unblock(tch)
            for g in gens0:  # run the generator tails (done_mt bookkeeping)
                for _ in g:
                    pass

            units = [(0, 1), (1, 1), (2, 1), (3, 1),
                     (0, 2), (1, 2), (2, 2), (3, 2),
                     (0, 3), (1, 3), (2, 3), (3, 3)]
            for i, (qb, p) in enumerate(units):
                need_pair(p)
                attn_unit(qb, p)
                op_unblock(qb, tail=(i == len(units) - 1))
            while step_filler():
                pass

    nc.finalize()
    return nc


_NC = None


def kernel(x, qkv_w, qkv_b, out_w, out_b, attn_mask):
    global _NC, LAST_RESULT
    if _NC is None:
        _NC = build()

    x = np.asarray(x, dtype=np.float32)
    qkv_w = np.asarray(qkv_w, dtype=np.float32)
    qkv_b = np.asarray(qkv_b, dtype=np.float32)
    out_w = np.asarray(out_w, dtype=np.float32)
    out_b = np.asarray(out_b, dtype=np.float32)
    mask = np.asarray(attn_mask).reshape(T, T)

    # only the diagonal 128-block of each key tile is ever masked
    band = np.empty((128, NKT, 128), dtype=ml_dtypes.bfloat16)
    for j in range(NKT):
        band[:, j, :] = mask[j * 128:(j + 1) * 128, j * 128:(j + 1) * 128].astype(
            ml_dtypes.bfloat16).T

    in_maps = []
    for c in range(8):
        b, hg = c % 4, c // 4
        h0 = hg * HPC
        # per-pair [q;k;v] row blocks of qkv_w, transposed; q pre-scaled by 1/8
        blocks = []
        bias_cols = np.empty((128, MT), dtype=np.float32)
        for p in range(PAIRS):
            r0 = (h0 + 2 * p) * D
            qrows = qkv_w[r0:r0 + 128] * 0.125
            krows = qkv_w[C + r0:C + r0 + 128]
            vrows = qkv_w[2 * C + r0:2 * C + r0 + 128]
            blocks += [qrows, krows, vrows]
            bias_cols[:, 3 * p + 0] = qkv_b[r0:r0 + 128] * 0.125
            bias_cols[:, 3 * p + 1] = qkv_b[C + r0:C + r0 + 128]
            bias_cols[:, 3 * p + 2] = qkv_b[2 * C + r0:2 * C + r0 + 128]
        wqkvT = np.concatenate(blocks, axis=0).T  # [C, MT*128]
        wtile = np.ascontiguousarray(
            wqkvT.reshape(CT, 128, MT, 128).transpose(1, 2, 0, 3)).astype(ml_dtypes.bfloat16)
        woT = np.ascontiguousarray(
            out_w[:, h0 * D:(h0 + HPC) * D].T).astype(ml_dtypes.bfloat16)
        bo = (np.tile(out_b, (128, 1)) if hg == 0
              else np.zeros((128, C), np.float32)).astype(np.float32)
        in_maps.append({
            "xT": np.ascontiguousarray(x[b].T).astype(ml_dtypes.bfloat16),
            "wtile": wtile,
            "bqkv": bias_cols,
            "woT": woT,
            "bo": bo,
            "maskband": band,
        })

    LAST_RESULT = run_bass_kernel_spmd(_NC, in_maps, core_ids=list(range(8)))
    res = LAST_RESULT.results
    out = np.empty((B, T, C), dtype=np.float32)
    for b in range(B):
        out[b] = (res[b]["out"].astype(np.float32)
                  + res[b + 4]["out"].astype(np.float32))
    return out
